# revision 2
# baseline (speedup 1.0000x reference)
"""GraphSAGE(max) 5-layer GNN on 8 Trainium2 NeuronCores.

v2: multi-queue non-transpose dma_gather. Key facts discovered on HW:
  * num_swdge_queues=4 gives 4 independent SWDGE queues, each with its own
    Q7 cpu pair for descriptor generation; round-robin dispatch reaches
    ~2.0 ns/row vs 7.9 ns/row serial. Queue q's dispatch blocks the engine
    while pair q is busy, so lead with queues 1-3 (queue 0's dispatch holds
    the engine either way).
  * transpose-mode gathers CORRUPT each other when concurrent (shared xbar
    write-combining state interleaves at descriptor granularity on the 16
    shared DMA engines); non-transpose (edge-major) gathers are clean.
  * Fold structure per (chunk, window): full-width slot "planes" tree-folded
    in edge-major with wide DVE ops; ragged tail slots packed densely,
    PE-transposed to feature-major, slot-folded; per-chunk aggregate
    PE-transposed back for the matmul epilogues (rows/pass ~107k vs 100k
    edges/core).
  * 5 AllGather segments with boundaries at locals 2048/4096/5632/6144 so
    window A = segs {0,1} exactly and the final segment is one small chunk;
    mid-pass triggers after each segment's producer chunks.
  * Passes 2 and 3 run all A-window calls first (gated only on segs 0-1 of
    the freshly gathered table), parking per-chunk partial aggregates, then
    the B phase + epilogues; pass 1 stays chunk-major for earliest stores.
Measured: 1.32 ms HW exec vs 2.59-2.60 ms for the v1 serial-gather kernel;
rel err 2.3e-3.
"""

import numpy as np

N_CORES = 8
N_NODES = 50000
F = 128
REAL_PER_CORE = N_NODES // N_CORES          # 6250
SPARE = 6
PER_CORE = REAL_PER_CORE + SPARE            # 6256
TOT_ROWS = N_CORES * PER_CORE               # 50048
WIN = 32768
WIN_B_BASE = TOT_ROWS - WIN                 # 17280
PAD_VAL = -60000.0
G_FULL = 512
CHUNKS = [G_FULL] * 12 + [PER_CORE - 12 * G_FULL]   # [512]*12 + [112]
CAP = 2048                                  # max gathered positions per call

# 5 segments, boundaries at locals 2048/4096/5632/6144: global 32768 falls
# exactly at the end of seg1, so window A = segs {0,1}; the final segment is
# tiny (chunk 12) so next-pass B-window calls wait on almost nothing.
SEGS = [(0, 2048), (2048, 4096), (4096, 5632), (5632, 6144), (6144, PER_CORE)]
SEG_BASE = [0]
for _lo, _hi in SEGS:
    SEG_BASE.append(SEG_BASE[-1] + N_CORES * (_hi - _lo))
SPARE_CHUNK = 7                             # spares at end of chunk 7
SPARE_LOC = 8 * G_FULL - SPARE              # 4090


def _glob_of(core, local):
    core = np.asarray(core)
    local = np.asarray(local)
    out = np.zeros(np.broadcast(core, local).shape, np.int64)
    for (lo, hi), b in zip(SEGS, SEG_BASE):
        m = (local >= lo) & (local < hi)
        out = np.where(m, b + core * (hi - lo) + (local - lo), out)
    return out


_P = np.arange(TOT_ROWS)
CORE_OF = np.zeros(TOT_ROWS, np.int64)
LOCAL_OF = np.zeros(TOT_ROWS, np.int64)
for (_lo, _hi), _b in zip(SEGS, SEG_BASE):
    _w = _hi - _lo
    _m = (_P >= _b) & (_P < _b + N_CORES * _w)
    CORE_OF = np.where(_m, (_P - _b) // _w, CORE_OF)
    LOCAL_OF = np.where(_m, _lo + (_P - _b) % _w, LOCAL_OF)

PAD_ROW = int(_glob_of(0, SPARE_LOC))
ZERO_ROW = PAD_ROW + 1
assert PAD_ROW + SPARE <= WIN and PAD_ROW >= WIN_B_BASE


def _block_classes():
    local = LOCAL_OF
    blk = np.minimum(local // G_FULL, len(CHUNKS) - 1)
    base = _glob_of(CORE_OF, blk * G_FULL)
    width = np.where(blk == len(CHUNKS) - 1, PER_CORE - 12 * G_FULL, G_FULL)
    a_elig = base + width <= WIN
    b_elig = base >= WIN_B_BASE
    assert (a_elig | b_elig).all()
    return a_elig, b_elig


def _edge_counts(src_pos, dst_pos, a_elig, b_elig):
    sa = a_elig[src_pos]
    sb = b_elig[src_pos]
    is_lo = sa & ~sb
    is_hi = sb & ~sa
    is_ov = sa & sb
    lo_cnt = np.bincount(dst_pos[is_lo], minlength=TOT_ROWS).astype(np.int64)
    ov_cnt = np.bincount(dst_pos[is_ov], minlength=TOT_ROWS).astype(np.int64)
    hi_cnt = np.bincount(dst_pos[is_hi], minlength=TOT_ROWS).astype(np.int64)
    empty = (lo_cnt + ov_cnt + hi_cnt) == 0
    lo2 = lo_cnt.copy()
    lo2[empty] = 1
    t = np.clip((hi_cnt + ov_cnt - lo2 + 1) // 2, 0, ov_cnt)
    nA = lo2 + t
    nB = hi_cnt + ov_cnt - t
    return nA, nB, t, empty


def _snake_perm(deg, edge_index):
    order = np.argsort(-deg, kind="stable")
    r = np.arange(N_NODES)
    rnd, p8 = r // N_CORES, r % N_CORES
    core = np.where(rnd % 2 == 0, p8, N_CORES - 1 - p8)
    local = np.where(rnd < SPARE_LOC, rnd, rnd + SPARE)
    pos_of_rank = _glob_of(core, local)
    pos = np.empty(N_NODES, np.int64)
    pos[order] = pos_of_rank

    # refine: sort real nodes within each (core, chunk) by max(nA, nB) desc
    # so both windows' prefix widths stay tight.
    src, dst = np.asarray(edge_index[0]), np.asarray(edge_index[1])
    a_elig, b_elig = _block_classes()
    nA, nB, _, _ = _edge_counts(pos[src], pos[dst], a_elig, b_elig)
    key = np.maximum(nA, nB)
    chunk_start = [sum(CHUNKS[:i]) for i in range(len(CHUNKS))]
    newpos = np.empty(TOT_ROWS, np.int64)
    for c in range(N_CORES):
        for ci, G in enumerate(CHUNKS):
            b0 = int(_glob_of(c, chunk_start[ci]))
            greal = G - SPARE if ci == SPARE_CHUNK else G
            seg = np.arange(b0, b0 + greal)
            o = np.argsort(-key[seg], kind="stable")
            newpos[seg[o]] = seg
            if greal < G:
                sp = np.arange(b0 + greal, b0 + G)
                newpos[sp] = sp
    pos = newpos[pos]

    node_by_pos = np.full(TOT_ROWS, -1, np.int64)
    node_by_pos[pos] = np.arange(N_NODES)
    return pos, node_by_pos


def _build_plan_and_indices(edge_index, pos):
    """Per (chunk, window): plane slots + ragged tail; split into calls.

    Call record: (w, chunk, col_off, n_pos, planes_here, tail)
      planes_here: number of full planes at the head of this call
      tail: list of (m_k,) widths for dense tail pieces in this call
    idx value semantics identical to v1 (window-relative table rows).
    """
    src, dst = np.asarray(edge_index[0]), np.asarray(edge_index[1])
    src_pos = pos[src]
    dst_pos = pos[dst]

    a_elig, b_elig = _block_classes()
    nA, nB, t, empty = _edge_counts(src_pos, dst_pos, a_elig, b_elig)

    chunk_start = [sum(CHUNKS[:i]) for i in range(len(CHUNKS))]

    def ceil128(x):
        return -(-x // 128) * 128

    # shared slot widths per (chunk, window): max over cores
    plan = []        # list of call records
    chunk_calls = [[] for _ in CHUNKS]  # per chunk: indices into plan
    col_off = 0
    for ci, G in enumerate(CHUNKS):
        off = chunk_start[ci]
        W = ceil128(G)
        for w, n_w in (("A", nA), ("B", nB)):
            mks = []
            K = 0
            for c in range(N_CORES):
                base = int(_glob_of(c, off))
                K = max(K, int(n_w[base:base + G].max()))
            for k in range(K):
                m = 0
                for c in range(N_CORES):
                    base = int(_glob_of(c, off))
                    nz = np.nonzero(n_w[base:base + G] >= k + 1)[0]
                    if nz.size:
                        m = max(m, int(nz[-1]) + 1)
                if m > 0:
                    mks.append(m)
            # planes: prefix of slots with m_k >= ~0.8*W
            thresh = max(W - 112, W * 3 // 4)
            U = 0
            while U < len(mks) and mks[U] >= thresh:
                U += 1
            tail = mks[U:]
            # split into calls: groups of planes (<= CAP/W) then tail
            plane_per_call = CAP // W
            u0 = 0
            while u0 < U:
                un = min(plane_per_call, U - u0)
                tail_here = []
                n_pos = un * W
                if u0 + un == U:
                    # append tail if it fits
                    tail_sz = ceil128(sum(tail))
                    if tail and n_pos + tail_sz <= CAP:
                        tail_here = tail
                        n_pos += tail_sz
                        tail = []
                plan.append((w, ci, col_off, n_pos, un, tail_here))
                chunk_calls[ci].append(len(plan) - 1)
                col_off += n_pos // 16
                u0 += un
            if tail or U == 0:
                # tail-only call (or window with no planes)
                n_pos = ceil128(sum(tail))
                if n_pos == 0:
                    n_pos = 128
                    tail = tail or [1]
                assert n_pos <= CAP, (ci, w, tail)
                plan.append((w, ci, col_off, n_pos, 0, tail))
                chunk_calls[ci].append(len(plan) - 1)
                col_off += n_pos // 16
    idx_cols = col_off

    # --- per-core idx arrays ---
    idx_arrays = []
    W_of = [ceil128(G) for G in CHUNKS]
    for c in range(N_CORES):
        e_mask = CORE_OF[dst_pos] == c
        sp = src_pos[e_mask]
        dl = LOCAL_OF[dst_pos[e_mask]].astype(np.int64)
        o2 = np.argsort(dl, kind="stable")
        sp, dl = sp[o2], dl[o2]
        lo_m = a_elig[sp] & ~b_elig[sp]
        hi_m = b_elig[sp] & ~a_elig[sp]
        ov_m = a_elig[sp] & b_elig[sp]
        ov_d = dl[ov_m]
        ov_rank = np.arange(ov_d.size) - np.searchsorted(ov_d, ov_d, "left")
        ov_toA = ov_rank < t[_glob_of(c, ov_d)]
        a_vals = np.concatenate([sp[lo_m], sp[ov_m][ov_toA]]).astype(np.int32)
        a_dsts = np.concatenate([dl[lo_m], ov_d[ov_toA]]).astype(np.int64)
        b_vals = (np.concatenate([sp[hi_m], sp[ov_m][~ov_toA]]).astype(np.int32)
                  - WIN_B_BASE)
        b_dsts = np.concatenate([dl[hi_m], ov_d[~ov_toA]]).astype(np.int64)

        def mk_window(wvals, wdsts):
            o3 = np.argsort(wdsts, kind="stable")
            wv, wd = wvals[o3], wdsts[o3]
            rank = np.arange(wd.size) - np.searchsorted(wd, wd, "left")
            return wv, wd, rank

        av, ad, ar = mk_window(a_vals, a_dsts)
        bv, bd, br = mk_window(b_vals, b_dsts)

        idx_arr = np.zeros((128, idx_cols), np.int16)
        # per chunk-window: cell matrix M[k, d]
        mats = {}
        for ci, G in enumerate(CHUNKS):
            off = chunk_start[ci]
            for wname, (wv, wd, wr), pad_idx in (
                ("A", (av, ad, ar), PAD_ROW),
                ("B", (bv, bd, br), PAD_ROW - WIN_B_BASE),
            ):
                # K for this chunk-window = max slots used in plan
                K = 0
                for pi in chunk_calls[ci]:
                    w2, _, _, _, un, tl = plan[pi]
                    if w2 == wname:
                        K += un + len(tl)
                M = np.full((max(K, 1), G), pad_idx, np.int32)
                m = (wd >= off) & (wd < off + G)
                sel = wr[m] < K
                M[wr[m][sel], wd[m][sel] - off] = wv[m][sel]
                if wname == "A":
                    g0 = int(_glob_of(c, off))
                    je = np.nonzero(empty[g0:g0 + G])[0]
                    M[0, je] = ZERO_ROW
                mats[(ci, wname)] = M

        # fill calls
        kpos = {}  # (ci, w) -> next slot index
        for (w2, ci, co, n_pos, un, tl) in plan:
            G = CHUNKS[ci]
            W = W_of[ci]
            pad_idx = PAD_ROW if w2 == "A" else PAD_ROW - WIN_B_BASE
            M = mats[(ci, w2)]
            k0 = kpos.get((ci, w2), 0)
            flat = np.full(n_pos, pad_idx, np.int32)
            p = 0
            for k in range(k0, k0 + un):
                flat[p:p + G] = M[k]
                p += W
            for i, mk in enumerate(tl):
                k = k0 + un + i
                if k < M.shape[0]:
                    flat[p:p + mk] = M[k, :mk]
                p += mk
            kpos[(ci, w2)] = k0 + un + len(tl)
            assert p <= n_pos
            blk = flat.astype(np.int16).reshape(n_pos // 16, 16).T
            for s in range(8):
                idx_arr[16 * s:16 * (s + 1), co:co + n_pos // 16] = blk
        idx_arrays.append(idx_arr)

    tot_pos = sum(p[3] for p in plan)
    stats = {"edges": int(src.size), "rows_per_pass": tot_pos,
             "idx_cols": idx_cols, "n_calls": len(plan)}
    return plan, chunk_calls, idx_arrays, idx_cols, stats


def _build_nc(plan, chunk_calls, idx_cols):
    import concourse.bass as bass
    import concourse.mybir as mybir
    import concourse.tile as tile
    from concourse import bacc
    from concourse.masks import make_identity

    fp16 = mybir.dt.float16
    f32 = mybir.dt.float32
    Relu = mybir.ActivationFunctionType.Relu
    Copy = mybir.ActivationFunctionType.Copy
    MAX = mybir.AluOpType.max

    nc = bacc.Bacc("TRN2", num_devices=N_CORES,
                   dynamic_dma_scratch_size=40960, num_swdge_queues=4)

    xtab = nc.dram_tensor("xtab", [TOT_ROWS, F], fp16, kind="ExternalInput")
    xloc = nc.dram_tensor("xloc", [F, PER_CORE], fp16, kind="ExternalInput")
    idx_in = nc.dram_tensor("idx", [128, idx_cols], mybir.dt.int16,
                            kind="ExternalInput")
    wpack = nc.dram_tensor("wpack", [F, 10 * F + 2], fp16,
                           kind="ExternalInput")
    bpack = nc.dram_tensor("bpack", [F, 7], f32, kind="ExternalInput")
    cpad = nc.dram_tensor("cpad", [SPARE, F], fp16, kind="ExternalInput")
    out2 = nc.dram_tensor("out2", [2, PER_CORE], f32, kind="ExternalOutput")

    a_mv = nc.dram_tensor("agin_mv", [PER_CORE, F], fp16)
    a_comb = nc.dram_tensor("agin_comb", [PER_CORE, 2 * F], fp16)
    # A-phase partial aggregates round-trip through DRAM (breaks per-chunk
    # WAR serialization on a shared SBUF tile)
    partials_d = nc.dram_tensor("partials", [3, F, PER_CORE], fp16)
    tab_mv = nc.dram_tensor("tab_mv", [TOT_ROWS, F], fp16, addr_space="Shared")
    tab_comb = nc.dram_tensor("tab_comb", [TOT_ROWS, 2 * F], fp16,
                              addr_space="Shared")

    LCOL = {"S": 0, "rt1": 2, "rt2": 4, "mv1": 6, "mv2": 8}
    BCOL = {"S": 0, "rt1": 1, "rt2": 2, "mv1": 3, "mv2": 4}
    chunk_off = [sum(CHUNKS[:i]) for i in range(len(CHUNKS))]
    W_of = [-(-G // 128) * 128 for G in CHUNKS]

    qctr = [0]

    def next_q():
        # q0 dispatches block the engine while its cpu pair is busy; lead
        # with the non-blocking queues so their gens prefetch.
        q = (1, 2, 3, 0)[qctr[0] % 4]
        qctr[0] += 1
        return q

    with tile.TileContext(nc) as tc:
        with (
            tc.tile_pool(name="persist", bufs=1) as pp,
            tc.tile_pool(name="gbE", bufs=8) as gbp,
            tc.tile_pool(name="gt", bufs=4) as gtp,       # tail fm staging
            tc.tile_pool(name="bigloc", bufs=5) as blp,
            tc.tile_pool(name="acce", bufs=3) as accep,
            tc.tile_pool(name="accf", bufs=4) as accfp,
            tc.tile_pool(name="aggf", bufs=4) as aggfp,
            tc.tile_pool(name="om", bufs=2) as omp,
            tc.tile_pool(name="rm", bufs=2) as rmp,
            tc.tile_pool(name="fb", bufs=2) as fbp,
            tc.tile_pool(name="psT", bufs=2, space="PSUM") as psTp,
            tc.tile_pool(name="psN", bufs=2, space="PSUM") as psNp,
            tc.tile_pool(name="psX", bufs=2, space="PSUM") as psXp,
            tc.tile_pool(name="psF", bufs=2, space="PSUM") as psFp,
        ):
            idx_t = pp.tile([128, idx_cols], mybir.dt.int16, tag="idx",
                            name="idx")
            c0 = plan[0][3] // 16
            nc.sync.dma_start(out=idx_t[:, :c0], in_=idx_in[:, :c0])
            nc.sync.dma_start(out=idx_t[:, c0:], in_=idx_in[:, c0:])
            w_t = pp.tile([F, 10 * F + 2], fp16, tag="w", name="w")
            nc.sync.dma_start(out=w_t[:], in_=wpack[:, :])
            b_t = pp.tile([F, 7], f32, tag="b", name="b")
            nc.sync.dma_start(out=b_t[:], in_=bpack[:, :])
            ident = pp.tile([F, F], fp16, tag="ident", name="ident")
            make_identity(nc, ident[:])

            locs = {}
            x_t = blp.tile([F, PER_CORE], fp16, tag="bigloc", name="xloc_t")
            nc.sync.dma_start(out=x_t[:], in_=xloc[:, :])
            loc_mv = blp.tile([F, PER_CORE], fp16, tag="bigloc",
                              name="loc_mv")
            for k in ("rt", "md", "pA"):
                locs[k] = blp.tile([F, PER_CORE], fp16, tag="bigloc",
                                   name=f"loc_{k}")
            # 6th tile on the 5-buf pool reuses x_t's buffer (dead after p1)
            locs["pB"] = blp.tile([F, PER_CORE], fp16, tag="bigloc",
                                  name="loc_pB")

            def wsl(lname):
                return (w_t[:, LCOL[lname] * F:(LCOL[lname] + 1) * F],
                        w_t[:, (LCOL[lname] + 1) * F:(LCOL[lname] + 2) * F],
                        b_t[:, BCOL[lname]:BCOL[lname] + 1])

            def do_call(pi, table, planes_n, acc_e, acc_fms):
                """One gather call + tree fold + tail processing.

                planes_n: 1 (pass1/2, es=128) or 2 (pass3, es=256)
                acc_e: edge-major accumulator [128, W/128 * es]
                acc_fms: list of feature-major accs [128, W] (1 or 2 streams)
                """
                (w2, ci, co, n_pos, un, tl) = plan[pi]
                es = planes_n * F
                W = W_of[ci]
                wblk = W // 128
                nblk = n_pos // 128
                winA = table[0:WIN, :]
                winB = table[WIN_B_BASE:WIN_B_BASE + WIN, :]
                gb = gbp.tile([128, nblk * es], fp16, tag="gbE", name="gbE",
                              padded_shape=[128, (CAP // 128) * 2 * F])
                nc.gpsimd.dma_gather(
                    gb[:, :nblk * es].rearrange("p (c e) -> p c e", e=es),
                    winA if w2 == "A" else winB,
                    idx_t[:, co:co + n_pos // 16],
                    n_pos, n_pos, es,
                    transpose=False, single_packet=False, queue_num=next_q(),
                )
                pw = wblk * es  # cols per plane
                # tree-fold planes 0..un-1 down to plane 0
                u = un
                while u > 1:
                    h = u // 2
                    nc.vector.tensor_tensor(
                        out=gb[:, :h * pw], in0=gb[:, :h * pw],
                        in1=gb[:, (u - h) * pw:u * pw], op=MAX)
                    u -= h
                if un > 0:
                    nc.vector.tensor_tensor(
                        out=acc_e[:, :pw], in0=acc_e[:, :pw],
                        in1=gb[:, :pw], op=MAX)
                # tail: PE-transpose each 128-position block to fm staging
                if tl:
                    t0 = un * wblk            # first tail block index
                    tn = nblk - t0            # tail blocks
                    gts = [gtp.tile([128, CAP], fp16, tag="gt", name="gt")
                           for _ in range(planes_n)]
                    for h in range(planes_n):
                        for g0 in range(0, tn, 8):
                            gn = min(8, tn - g0)
                            psX = psXp.tile([128, 1024], fp16, tag="psX",
                                            name="psX")
                            for b in range(g0, g0 + gn):
                                nc.tensor.transpose(
                                    out=psX[:, (b - g0) * 128:
                                            (b - g0) * 128 + 128],
                                    in_=gb[:, (t0 + b) * es + h * F:
                                           (t0 + b) * es + h * F + F],
                                    identity=ident[:])
                            nc.scalar.activation(
                                out=gts[h][:, g0 * 128:(g0 + gn) * 128],
                                in_=psX[:, :gn * 128],
                                func=Copy, bias=0.0, scale=1.0)
                    # slot folds from fm staging
                    off = 0
                    for mk in tl:
                        for h in range(planes_n):
                            nc.vector.tensor_tensor(
                                out=acc_fms[h][:, :mk],
                                in0=acc_fms[h][:, :mk],
                                in1=gts[h][:, off:off + mk], op=MAX)
                        off += mk

            def chunk_agg(ci, planes_n, acc_e, acc_fms, targets=None):
                """Transpose acc_e to fm, merge tails -> agg tiles.

                targets: optional [(tile, col_off)] per stream to write into
                (persistent per-chunk slices) instead of fresh pool tiles.
                """
                W = W_of[ci]
                wblk = W // 128
                es = planes_n * F
                aggs = []
                G = CHUNKS[ci]
                for h in range(planes_n):
                    if targets is None:
                        agg, oo = aggfp.tile([128, G_FULL], fp16, tag="aggf",
                                             name="aggf"), 0
                        Wc = W
                    else:
                        agg, oo = targets[h]
                        Wc = min(W, G)
                    for b in range(wblk):
                        bw = min(128, Wc - b * 128)
                        psN = psNp.tile([128, 128], fp16, tag="psN",
                                        name="psN")
                        nc.tensor.transpose(
                            out=psN[:, :],
                            in_=acc_e[:, b * es + h * F:b * es + h * F + F],
                            identity=ident[:])
                        nc.scalar.activation(
                            out=agg[:, oo + b * 128:oo + b * 128 + bw],
                            in_=psN[:, :bw], func=Copy, bias=0.0, scale=1.0)
                    nc.vector.tensor_tensor(out=agg[:, oo:oo + Wc],
                                            in0=agg[:, oo:oo + Wc],
                                            in1=acc_fms[h][:, :Wc], op=MAX)
                    aggs.append(agg)
                return aggs

            def mm_epilogue(ci, off, agg, xT, lname, out_tile, out_off=None):
                G = CHUNKS[ci]
                oo = off if out_off is None else out_off
                wl, wr, bias = wsl(lname)
                psT = psTp.tile([F, G_FULL], f32, tag="psT", name="psT")
                nc.tensor.matmul(out=psT[:, :G], lhsT=wl, rhs=agg[:, :G],
                                 start=True, stop=False)
                nc.tensor.matmul(out=psT[:, :G], lhsT=wr,
                                 rhs=xT[:, off:off + G],
                                 start=False, stop=True)
                nc.scalar.activation(out=out_tile[:, oo:oo + G],
                                     in_=psT[:, :G], func=Relu, bias=bias,
                                     scale=1.0)

            def to_node_major(ci, off, out_tile, dst_dram, dcol):
                G = CHUNKS[ci]
                ngroups = -(-G // 128)
                om = omp.tile([128, 4 * 128], fp16, tag="om", name="om")
                for g in range(ngroups):
                    gw = min(128, G - g * 128)
                    psN = psNp.tile([128, 128], fp16, tag="psN", name="psN")
                    nc.tensor.transpose(
                        out=psN[:gw, :],
                        in_=out_tile[:, off + g * 128:off + g * 128 + gw],
                        identity=ident[:])
                    nc.scalar.activation(out=om[:gw, g * 128:(g + 1) * 128],
                                         in_=psN[:gw, :], func=Copy,
                                         bias=0.0, scale=1.0)
                if ci == SPARE_CHUNK:
                    nc.sync.dma_start(
                        out=om[SPARE_LOC % 128:SPARE_LOC % 128 + SPARE,
                               384:512], in_=cpad[:, :])
                if G == G_FULL:
                    nc.sync.dma_start(
                        out=dst_dram[off:off + G, dcol:dcol + 128].rearrange(
                            "(g p) f -> p g f", p=128),
                        in_=om[:].rearrange("p (g f) -> p g f", g=4))
                else:
                    nc.sync.dma_start(out=dst_dram[off:off + G,
                                                   dcol:dcol + 128],
                                      in_=om[:G, 0:128])

            def allgather_seg(a_in, tab, seg):
                lo, hi = SEGS[seg]
                b = SEG_BASE[seg]
                nc.gpsimd.collective_compute(
                    "AllGather", mybir.AluOpType.bypass,
                    replica_groups=[list(range(N_CORES))],
                    ins=[a_in[lo:hi, :]],
                    outs=[tab[b:b + N_CORES * (hi - lo), :]])

            # seg s's producer chunks end at {3,7,10,11}; dispatch each
            # trigger ~2 chunks later so its wait doesn't stall the gather
            # dispatch stream while the store chain drains.
            TRIG_AT = {3: (0,), 7: (1,), 10: (2,), 11: (3,)}

            def fresh_accs(ci, planes_n):
                W = W_of[ci]
                wblk = W // 128
                es = planes_n * F
                acc_e = accep.tile([128, 4 * 2 * F], fp16, tag="acce",
                                   name="acce")
                nc.vector.memset(acc_e[:, :wblk * es], PAD_VAL)
                acc_fms = [accfp.tile([128, G_FULL], fp16, tag="accf",
                                      name="accf")
                           for _ in range(planes_n)]
                for a in acc_fms:
                    nc.vector.memset(a[:, :W], PAD_VAL)
                return acc_e, acc_fms

            def do_pass(table, planes_n, epilogue, head_trigger=None,
                        mid_trigger=None):
                # head_trigger = AllGather of the table's last segment; every
                # B-window call reads it, so it must dispatch before the first
                # B call (engine program order) but after chunk-0 A calls so
                # the A gens prefetch behind the collective's wait.
                for ci in range(len(CHUNKS)):
                    acc_e, acc_fms = fresh_accs(ci, planes_n)
                    acalls = [pi for pi in chunk_calls[ci]
                              if plan[pi][0] == "A"]
                    bcalls = [pi for pi in chunk_calls[ci]
                              if plan[pi][0] == "B"]
                    for pi in acalls:
                        do_call(pi, table, planes_n, acc_e, acc_fms)
                    if ci == 0 and head_trigger is not None:
                        head_trigger()
                    for pi in bcalls:
                        do_call(pi, table, planes_n, acc_e, acc_fms)
                    aggs = chunk_agg(ci, planes_n, acc_e, acc_fms)
                    epilogue(ci, chunk_off[ci], aggs)
                    if mid_trigger is not None:
                        for s in TRIG_AT.get(ci, ()):
                            mid_trigger(s)

            def do_pass_split(table, planes_n, epilogue, head_trigger,
                              partials, mid_trigger=None):
                """Run every chunk's A-window calls first (they only need the
                table's first two segments), parking per-chunk partial
                aggregates in `partials` slices, then the B phase with
                epilogues (and segment triggers) per chunk."""
                for ci in range(len(CHUNKS)):
                    acc_e, acc_fms = fresh_accs(ci, planes_n)
                    for pi in chunk_calls[ci]:
                        if plan[pi][0] == "A":
                            do_call(pi, table, planes_n, acc_e, acc_fms)
                    if ci == 0 and head_trigger is not None:
                        head_trigger()
                    chunk_agg(ci, planes_n, acc_e, acc_fms,
                              targets=[(p, chunk_off[ci]) for p in partials])
                for ci in range(len(CHUNKS)):
                    off = chunk_off[ci]
                    W = W_of[ci]
                    acc_e, acc_fms = fresh_accs(ci, planes_n)
                    for pi in chunk_calls[ci]:
                        if plan[pi][0] == "B":
                            do_call(pi, table, planes_n, acc_e, acc_fms)
                    aggs = chunk_agg(ci, planes_n, acc_e, acc_fms)
                    Wc = min(W, CHUNKS[ci])
                    for h, p in enumerate(partials):
                        nc.vector.tensor_tensor(
                            out=aggs[h][:, :Wc], in0=aggs[h][:, :Wc],
                            in1=p[:, off:off + Wc], op=MAX)
                    epilogue(ci, off, aggs)
                    if mid_trigger is not None:
                        for s in TRIG_AT.get(ci, ()):
                            mid_trigger(s)

            # ---- pass 1: agg(x) -> mv ----
            def epi1(ci, off, aggs):
                mm_epilogue(ci, off, aggs[0], x_t, "S", loc_mv)
                to_node_major(ci, off, loc_mv, a_mv, 0)

            do_pass(xtab, 1, epi1,
                    mid_trigger=lambda s: allgather_seg(a_mv, tab_mv, s))
            # (the final segment, 4, is head-triggered by the next pass)

            # ---- pass 2: agg(mv) -> rt, md ----
            def epi2(ci, off, aggs):
                mm_epilogue(ci, off, aggs[0], loc_mv, "rt1", locs["rt"])
                mm_epilogue(ci, off, aggs[0], loc_mv, "mv1", locs["md"])
                to_node_major(ci, off, locs["rt"], a_comb, 0)
                to_node_major(ci, off, locs["md"], a_comb, 128)

            do_pass_split(tab_mv, 1, epi2,
                          head_trigger=lambda: allgather_seg(a_mv, tab_mv, 4),
                          partials=[locs["pA"]],
                          mid_trigger=lambda s: allgather_seg(a_comb,
                                                              tab_comb, s))

            # ---- pass 3: agg([rt|md]) -> r2, m2, finals ----
            rtw = w_t[:, 10 * F:10 * F + 1]
            mvw = w_t[:, 10 * F + 1:10 * F + 2]
            rtb = b_t[0:1, 5:6]
            mvb = b_t[0:1, 6:7]

            def epi3(ci, off, aggs):
                G = CHUNKS[ci]
                for row, (wv, bv, lname, loc) in enumerate(
                    ((rtw, rtb, "rt2", "rt"), (mvw, mvb, "mv2", "md"))
                ):
                    rm = rmp.tile([F, G_FULL], fp16, tag="rm", name="rm")
                    mm_epilogue(ci, off, aggs[row], locs[loc], lname, rm,
                                out_off=0)
                    psF = psFp.tile([1, G_FULL], f32, tag="psF", name="psF")
                    nc.tensor.matmul(out=psF[:1, :G], lhsT=wv,
                                     rhs=rm[:, :G],
                                     start=True, stop=True)
                    fbuf = fbp.tile([1, G_FULL], f32, tag="fbuf", name="fbuf")
                    nc.vector.tensor_scalar(
                        out=fbuf[0:1, :G], in0=psF[:1, :G],
                        scalar1=bv, scalar2=None, op0=mybir.AluOpType.add)
                    nc.sync.dma_start(out=out2[row:row + 1, off:off + G],
                                      in_=fbuf[0:1, :G])

            do_pass_split(tab_comb, 2, epi3,
                          head_trigger=lambda: allgather_seg(a_comb, tab_comb,
                                                             4),
                          partials=[locs["pA"], locs["pB"]])

    nc.finalize()
    return nc


_CACHE = {}
_TRACE = False
_LAST_RESULT = None


def kernel(x, edge_index, shared_Wl, shared_b, shared_Wr,
           rt1_Wl, rt1_b, rt1_Wr, rt2_Wl, rt2_b, rt2_Wr, rt3_W, rt3_b,
           mv1_Wl, mv1_b, mv1_Wr, mv2_Wl, mv2_b, mv2_Wr, mv3_W, mv3_b):
    from concourse.bass_utils import run_bass_kernel_spmd

    x = np.asarray(x)
    edge_index = np.asarray(edge_index)

    key = hash(edge_index.tobytes())
    if key not in _CACHE:
        deg = np.bincount(edge_index[1], minlength=N_NODES)
        pos, node_by_pos = _snake_perm(deg, edge_index)
        plan, chunk_calls, idx_arrays, idx_cols, stats = \
            _build_plan_and_indices(edge_index, pos)
        nc = _build_nc(plan, chunk_calls, idx_cols)
        _CACHE[key] = (pos, node_by_pos, idx_arrays, stats, nc)
    pos, node_by_pos, idx_arrays, stats, nc = _CACHE[key]

    xtab = np.empty((TOT_ROWS, F), np.float16)
    real = node_by_pos >= 0
    xtab[real] = x[node_by_pos[real]].astype(np.float16)
    for c in range(N_CORES):
        b0 = int(_glob_of(c, SPARE_LOC))
        xtab[b0:b0 + SPARE] = PAD_VAL
        xtab[b0 + 1] = 0.0

    def t16(w):
        return np.ascontiguousarray(np.asarray(w).T.astype(np.float16))

    wpack = np.concatenate(
        [t16(shared_Wl), t16(shared_Wr), t16(rt1_Wl), t16(rt1_Wr),
         t16(rt2_Wl), t16(rt2_Wr), t16(mv1_Wl), t16(mv1_Wr),
         t16(mv2_Wl), t16(mv2_Wr), t16(rt3_W), t16(mv3_W)], axis=1)
    bpack = np.zeros((F, 7), np.float32)
    for i, b in enumerate((shared_b, rt1_b, rt2_b, mv1_b, mv2_b)):
        bpack[:, i] = np.asarray(b, np.float32)
    bpack[0, 5] = float(np.asarray(rt3_b).reshape(-1)[0])
    bpack[0, 6] = float(np.asarray(mv3_b).reshape(-1)[0])

    cpad_arr = np.full((SPARE, F), PAD_VAL, np.float16)
    cpad_arr[1] = 0.0

    in_maps = []
    loc_globs = []
    for c in range(N_CORES):
        loc_glob = np.concatenate([
            np.arange(b + c * (hi - lo), b + (c + 1) * (hi - lo))
            for (lo, hi), b in zip(SEGS, SEG_BASE)])
        loc_globs.append(loc_glob)
        xloc = np.ascontiguousarray(xtab[loc_glob].T)
        xloc[:, SPARE_LOC:SPARE_LOC + SPARE] = 0
        in_maps.append({
            "xtab": xtab, "xloc": xloc, "idx": idx_arrays[c],
            "wpack": wpack, "bpack": bpack, "cpad": cpad_arr,
        })

    global _LAST_RESULT
    res = run_bass_kernel_spmd(nc, in_maps, core_ids=list(range(N_CORES)),
                               trace=_TRACE)
    _LAST_RESULT = res

    rtAngle = np.empty(N_NODES, np.float32)
    moveDis = np.empty(N_NODES, np.float32)
    for c in range(N_CORES):
        o = res.results[c]["out2"]
        nodes = node_by_pos[loc_globs[c]]
        m = nodes >= 0
        rtAngle[nodes[m]] = o[0][m]
        moveDis[nodes[m]] = o[1][m]
    return (rtAngle, moveDis)
